# Initial kernel scaffold
#
"""Bass/Trainium2 kernel for nn_Channel_attention (bottom-16 channel gather).

reference semantics (per sample b):
    weight = mean(x[b], axis=(H, W))           # [C]
    idx    = argsort(weight)[:16]              # ascending pooled value
    out[b] = x[b, idx]                         # [16, H, W]

Strategy: pure data parallel, B=16 sharded 2 samples per core over 8 cores.
Per core (x shard viewed as [512, 16384] = [(sample, channel), H*W]):
  1. Stream [128ch, 2048] tiles, DVE reduce_add -> per-channel partial sums.
     Load DMAs alternate between the sync and scalar HWDGE queues.
  2. Per sample (pipelined so sample 0's tail hides under sample 1's loads):
     negate sums on DVE, PE-transpose into a [1, 256] row, two rounds of
     max8/max_index/match_replace -> bottom-16 channel indices in ascending
     order of pooled sum (argsort of sum == argsort of mean).
  3. Per max8 round, expand its 8 indices to 64 gather-row indices
     (idx*8 + subrow) with two tiny PE matmuls, SWDGE indirect-gather
     [64, 2048] (8 KiB rows are the line-rate descriptor size), and store
     contiguously; round 1's gather+store runs while round 2 still selects.
"""

import sys

if "/opt/trn_rl_repo" not in sys.path:
    sys.path.insert(0, "/opt/trn_rl_repo")

import numpy as np

from concourse import bacc, mybir, tile
from concourse.bass import IndirectOffsetOnAxis
from concourse.bass_utils import run_bass_kernel_spmd
from concourse.masks import make_identity

N_CORES = 8
B, C, H, W = 16, 256, 128, 128
K = 16
BPC = B // N_CORES          # samples per core = 2
E = H * W                   # 16384 elems per channel
GR = 8                      # gather sub-rows per channel (8 x 8KiB)
GP = K * GR                 # gather tile partitions
GW = E // GR                # gather row width (elems)
LOG2GR = GR.bit_length() - 1
ROWS = BPC * C              # 512 channel rows per core

f32 = mybir.dt.float32
i32 = mybir.dt.int32
u32 = mybir.dt.uint32
X = mybir.AxisListType.X
Alu = mybir.AluOpType

# chunk widths per (sample, half); last half of the last sample ends with
# small chunks so the final reduce exits quickly after the last load lands
CHUNKS = [2048] * 8
CHUNKS_LAST = [2048] * 5 + [1024] * 6

_cache = {}


def _build():
    nc = bacc.Bacc("TRN2", target_bir_lowering=False, debug=False,
                   num_devices=N_CORES)
    x_d = nc.dram_tensor("x", [ROWS, E], f32, kind="ExternalInput")
    y_d = nc.dram_tensor("y", [BPC * K * GR, GW], f32,
                         kind="ExternalOutput")

    with tile.TileContext(nc) as tc:
        with (
            tc.tile_pool(name="load", bufs=20) as load_pool,
            tc.tile_pool(name="small", bufs=1) as small,
            tc.tile_pool(name="gather", bufs=1) as gather_pool,
            tc.tile_pool(name="psum", bufs=1, space="PSUM") as psum,
        ):
            # ---- constants (no deps; scheduler fills gaps with these) ----
            ident = small.tile([128, 128], f32)
            make_identity(nc, ident[:])

            e_i = small.tile([K, GP], i32)
            nc.gpsimd.iota(out=e_i[:], pattern=[[1, GP]], base=0,
                           channel_multiplier=0)
            nc.vector.tensor_scalar(out=e_i[:], in0=e_i[:], scalar1=LOG2GR,
                                    scalar2=None, op0=Alu.arith_shift_right)
            e_f = small.tile([K, GP], f32)
            nc.vector.tensor_copy(e_f[:], e_i[:])
            col_i = small.tile([K, 1], i32)
            nc.gpsimd.iota(out=col_i[:], pattern=[[1, 1]], base=0,
                           channel_multiplier=1)
            col_f = small.tile([K, 1], f32)
            nc.vector.tensor_copy(col_f[:], col_i[:])
            e_mat = small.tile([K, GP], f32)
            nc.vector.tensor_scalar(out=e_mat[:], in0=e_f[:], scalar1=col_f[:],
                                    scalar2=None, op0=Alu.is_equal)

            pp = small.tile([GP, 1], i32)
            nc.gpsimd.iota(out=pp[:], pattern=[[1, 1]], base=0,
                           channel_multiplier=1)
            nc.vector.tensor_scalar(out=pp[:], in0=pp[:], scalar1=GR - 1,
                                    scalar2=None, op0=Alu.bitwise_and)
            a7f = small.tile([GP, 1], f32)
            nc.vector.tensor_copy(a7f[:], pp[:])

            xg = x_d[:].rearrange("r (u e) -> (r u) e", u=GR)
            dma_engines = [nc.sync, nc.scalar]
            n_dma = 0

            # ---- per-sample pipeline ----
            for s in range(BPC):
                ncols = 0
                chunk_lists = []
                for h in range(2):
                    cl = CHUNKS_LAST if (s == BPC - 1 and h == 1) else CHUNKS
                    chunk_lists.append(cl)
                    ncols = max(ncols, len(cl))
                partials = small.tile([128, 2 * ncols], f32, tag=f"partials{s}")

                sums = small.tile([128, 2], f32, tag=f"sums{s}")
                psum_w = psum.tile([1, C], f32, tag=f"psw{s}")
                w_neg = small.tile([1, C], f32, tag=f"wneg{s}")
                for h in range(2):
                    base = s * C + h * 128
                    off = 0
                    cl = chunk_lists[h]
                    for j, cw in enumerate(cl):
                        t = load_pool.tile([128, 2048], f32)
                        eng = dma_engines[n_dma % 2]
                        n_dma += 1
                        eng.dma_start(out=t[:, 0:cw],
                                      in_=x_d[base:base + 128, off:off + cw])
                        nc.vector.reduce_sum(
                            out=partials[:, h * ncols + j:h * ncols + j + 1],
                            in_=t[:, 0:cw], axis=X)
                        off += cw

                    # this half's sums + transpose, while the other half
                    # (or the next sample) is still streaming
                    nc.vector.reduce_sum(
                        out=sums[:, h:h + 1],
                        in_=partials[:, h * ncols:h * ncols + len(cl)],
                        axis=X, negate=True)
                    nc.tensor.matmul(out=psum_w[:, h * 128:(h + 1) * 128],
                                     lhsT=sums[:, h:h + 1], rhs=ident[:],
                                     start=True, stop=True)
                    nc.vector.tensor_copy(w_neg[:, h * 128:(h + 1) * 128],
                                          psum_w[:, h * 128:(h + 1) * 128])

                # bottom-16 via two rounds of max8 on -sums; ranks 0-7
                # gather+store as soon as round 1's indices land, while
                # round 2 is still running on DVE
                m1 = small.tile([1, 8], f32, tag=f"m1_{s}")
                m2 = small.tile([1, 8], f32, tag=f"m2_{s}")
                idx_u = small.tile([1, K], u32, tag=f"idxu{s}")
                w_rep = small.tile([1, C], f32, tag=f"wrep{s}")
                half = GP // 2
                st_eng = [nc.sync, nc.scalar]
                g = gather_pool.tile([GP, GW], f32, tag=f"g{s}")

                def expand_and_gather(r, m):
                    # gather-row index for tile partition p (p in [0, 64)):
                    # (s*C + idx[8r + (p>>3)])*GR + (p & (GR-1))
                    idx_f = small.tile([1, 8], f32, tag=f"idxf{s}_{r}")
                    nc.vector.tensor_copy(idx_f[:], idx_u[:, 8 * r:8 * r + 8])
                    psum_t = psum.tile([8, 1], f32, tag=f"pst{s}")
                    nc.tensor.matmul(out=psum_t[:], lhsT=idx_f[:],
                                     rhs=ident[0:1, 0:1], start=True,
                                     stop=True)
                    idx_t = small.tile([8, 1], f32, tag=f"idxt{s}_{r}")
                    nc.vector.tensor_copy(idx_t[:], psum_t[:])
                    psum_e = psum.tile([half, 1], f32, tag=f"pse{s}")
                    nc.tensor.matmul(out=psum_e[:], lhsT=e_mat[0:8, 0:half],
                                     rhs=idx_t[:], start=True, stop=True)
                    idx64_f = small.tile([half, 1], f32, tag=f"i64f{s}_{r}")
                    nc.vector.tensor_scalar(out=idx64_f[:], in0=psum_e[:],
                                            scalar1=float(GR),
                                            scalar2=float(s * C * GR),
                                            op0=Alu.mult, op1=Alu.add)
                    idx64_i = small.tile([half, 1], i32, tag=f"i64i{s}_{r}")
                    nc.vector.tensor_tensor(out=idx64_i[:], in0=idx64_f[:],
                                            in1=a7f[0:half, :], op=Alu.add)
                    # round 0 lands on partitions 0-63 (SDMA engines 0-7),
                    # round 1 on 64-127 (engines 8-15) so the two gathers'
                    # partition-bound descriptors run on disjoint engines
                    nc.gpsimd.indirect_dma_start(
                        out=g[r * half:(r + 1) * half, :], out_offset=None,
                        in_=xg,
                        in_offset=IndirectOffsetOnAxis(ap=idx64_i[:], axis=0))
                    st_eng[r].dma_start(
                        out=y_d[s * GP + r * half:s * GP + (r + 1) * half, :],
                        in_=g[r * half:(r + 1) * half, :])

                nc.vector.max(out=m1[:], in_=w_neg[:])
                nc.vector.max_index(out=idx_u[:, 0:8], in_max=m1[:],
                                    in_values=w_neg[:])
                expand_and_gather(0, m1)
                nc.vector.match_replace(out=w_rep[:], in_to_replace=m1[:],
                                        in_values=w_neg[:], imm_value=-1e38)
                nc.vector.max(out=m2[:], in_=w_rep[:])
                nc.vector.max_index(out=idx_u[:, 8:16], in_max=m2[:],
                                    in_values=w_rep[:])
                expand_and_gather(1, m2)

    nc.compile()
    return nc


def get_nc():
    if "nc" not in _cache:
        _cache["nc"] = _build()
    return _cache["nc"]


def make_in_maps(x: np.ndarray) -> list[dict[str, np.ndarray]]:
    x = np.ascontiguousarray(np.asarray(x, dtype=np.float32))
    assert x.shape == (B, C, H, W)
    return [{"x": x[c * BPC:(c + 1) * BPC].reshape(ROWS, E)}
            for c in range(N_CORES)]


def assemble(results: list[dict[str, np.ndarray]]) -> np.ndarray:
    out = np.empty((B, K, H, W), dtype=np.float32)
    for c in range(N_CORES):
        out[c * BPC:(c + 1) * BPC] = results[c]["y"].reshape(BPC, K, H, W)
    return out


def kernel(x: np.ndarray) -> np.ndarray:
    nc = get_nc()
    res = run_bass_kernel_spmd(nc, make_in_maps(x), list(range(N_CORES)))
    return assemble(res.results)



# revision 1
# speedup vs baseline: 1.1413x; 1.1413x over previous
"""Bass/Trainium2 kernel for nn_Channel_attention (bottom-16 channel gather).

reference semantics (per sample b):
    weight = mean(x[b], axis=(H, W))           # [C]
    idx    = argsort(weight)[:16]              # ascending pooled value
    out[b] = x[b, idx]                         # [16, H, W]

Strategy: pure data parallel, B=16 sharded 2 samples per core over 8 cores.
Per core (x shard viewed as [512, 16384] = [(sample, channel), H*W]):
  1. Stream [128ch, 2048] tiles, DVE reduce_add -> per-channel partial sums.
     Load DMAs alternate between the sync and scalar HWDGE queues.
  2. Per sample (pipelined so sample 0's tail hides under sample 1's loads):
     negate sums on DVE, PE-transpose into a [1, 256] row, two rounds of
     max8/max_index/match_replace -> bottom-16 channel indices in ascending
     order of pooled sum (argsort of sum == argsort of mean).
  3. Per max8 round, expand its 8 indices to 64 gather-row indices
     (idx*8 + subrow) with two tiny PE matmuls, SWDGE indirect-gather
     [64, 2048] (8 KiB rows are the line-rate descriptor size), and store
     contiguously; round 1's gather+store runs while round 2 still selects.
"""

import sys

if "/opt/trn_rl_repo" not in sys.path:
    sys.path.insert(0, "/opt/trn_rl_repo")

import numpy as np

from concourse import bacc, mybir, tile
from concourse.bass import IndirectOffsetOnAxis
from concourse.bass_utils import run_bass_kernel_spmd
from concourse.masks import make_identity

N_CORES = 8
B, C, H, W = 16, 256, 128, 128
K = 16
BPC = B // N_CORES          # samples per core = 2
E = H * W                   # 16384 elems per channel
GR = 8                      # gather sub-rows per channel (8 x 8KiB)
GP = K * GR                 # gather tile partitions
GW = E // GR                # gather row width (elems)
LOG2GR = GR.bit_length() - 1
ROWS = BPC * C              # 512 channel rows per core

f32 = mybir.dt.float32
i32 = mybir.dt.int32
u32 = mybir.dt.uint32
X = mybir.AxisListType.X
Alu = mybir.AluOpType

# chunk widths per (sample, half); last half of the last sample ends with
# small chunks so the final reduce exits quickly after the last load lands
CHUNKS = [2048] * 8
CHUNKS_LAST = [2048] * 5 + [1024] * 6

_cache = {}


def _build():
    nc = bacc.Bacc("TRN2", target_bir_lowering=False, debug=False,
                   num_devices=N_CORES)
    x_d = nc.dram_tensor("x", [ROWS, E], f32, kind="ExternalInput")
    y_d = nc.dram_tensor("y", [BPC * K * GR, GW], f32,
                         kind="ExternalOutput")

    with tile.TileContext(nc) as tc:
        with (
            tc.tile_pool(name="load", bufs=20) as load_pool,
            tc.tile_pool(name="small", bufs=1) as small,
            tc.tile_pool(name="gather", bufs=1) as gather_pool,
            tc.tile_pool(name="psum", bufs=1, space="PSUM") as psum,
        ):
            # ---- constants (no deps; scheduler fills gaps with these) ----
            ident = small.tile([128, 128], f32)
            make_identity(nc, ident[:])

            e_i = small.tile([K, GP], i32)
            nc.gpsimd.iota(out=e_i[:], pattern=[[1, GP]], base=0,
                           channel_multiplier=0)
            nc.vector.tensor_scalar(out=e_i[:], in0=e_i[:], scalar1=LOG2GR,
                                    scalar2=None, op0=Alu.arith_shift_right)
            e_f = small.tile([K, GP], f32)
            nc.vector.tensor_copy(e_f[:], e_i[:])
            col_i = small.tile([K, 1], i32)
            nc.gpsimd.iota(out=col_i[:], pattern=[[1, 1]], base=0,
                           channel_multiplier=1)
            col_f = small.tile([K, 1], f32)
            nc.vector.tensor_copy(col_f[:], col_i[:])
            e_mat = small.tile([K, GP], f32)
            nc.vector.tensor_scalar(out=e_mat[:], in0=e_f[:], scalar1=col_f[:],
                                    scalar2=None, op0=Alu.is_equal)

            pp = small.tile([GP, 1], i32)
            nc.gpsimd.iota(out=pp[:], pattern=[[1, 1]], base=0,
                           channel_multiplier=1)
            nc.vector.tensor_scalar(out=pp[:], in0=pp[:], scalar1=GR - 1,
                                    scalar2=None, op0=Alu.bitwise_and)
            a7f = small.tile([GP, 1], f32)
            nc.vector.tensor_copy(a7f[:], pp[:])

            xg = x_d[:].rearrange("r (u e) -> (r u) e", u=GR)
            dma_engines = [nc.sync, nc.scalar]
            n_dma = 0

            # ---- per-sample pipeline ----
            for s in range(BPC):
                ncols = 0
                chunk_lists = []
                for h in range(2):
                    cl = CHUNKS_LAST if (s == BPC - 1 and h == 1) else CHUNKS
                    chunk_lists.append(cl)
                    ncols = max(ncols, len(cl))
                partials = small.tile([128, 2 * ncols], f32, tag=f"partials{s}")

                sums = small.tile([128, 2], f32, tag=f"sums{s}")
                psum_w = psum.tile([1, C], f32, tag=f"psw{s}")
                w_neg = small.tile([1, C], f32, tag=f"wneg{s}")
                for h in range(2):
                    base = s * C + h * 128
                    off = 0
                    cl = chunk_lists[h]
                    for j, cw in enumerate(cl):
                        t = load_pool.tile([128, 2048], f32)
                        eng = dma_engines[n_dma % 2]
                        n_dma += 1
                        eng.dma_start(out=t[:, 0:cw],
                                      in_=x_d[base:base + 128, off:off + cw])
                        nc.vector.reduce_sum(
                            out=partials[:, h * ncols + j:h * ncols + j + 1],
                            in_=t[:, 0:cw], axis=X)
                        off += cw

                    # this half's sums + transpose, while the other half
                    # (or the next sample) is still streaming
                    nc.vector.reduce_sum(
                        out=sums[:, h:h + 1],
                        in_=partials[:, h * ncols:h * ncols + len(cl)],
                        axis=X, negate=True)
                    nc.tensor.matmul(out=psum_w[:, h * 128:(h + 1) * 128],
                                     lhsT=sums[:, h:h + 1], rhs=ident[:],
                                     start=True, stop=True)
                    nc.vector.tensor_copy(w_neg[:, h * 128:(h + 1) * 128],
                                          psum_w[:, h * 128:(h + 1) * 128])

                # bottom-16 via two rounds of max8 on -sums; ranks 0-7
                # gather+store as soon as round 1's indices land, while
                # round 2 is still running on DVE
                m1 = small.tile([1, 8], f32, tag=f"m1_{s}")
                m2 = small.tile([1, 8], f32, tag=f"m2_{s}")
                idx_u = small.tile([1, K], u32, tag=f"idxu{s}")
                w_rep = small.tile([1, C], f32, tag=f"wrep{s}")
                half = GP // 2
                st_eng = [nc.sync, nc.scalar]
                g = gather_pool.tile([GP, GW], f32, tag=f"g{s}")

                def expand_and_gather(r, m):
                    # gather-row index for tile partition p (p in [0, 64)):
                    # (s*C + idx[8r + (p>>3)])*GR + (p & (GR-1))
                    idx_f = small.tile([1, 8], f32, tag=f"idxf{s}_{r}")
                    nc.vector.tensor_copy(idx_f[:], idx_u[:, 8 * r:8 * r + 8])
                    psum_t = psum.tile([8, 1], f32, tag=f"pst{s}")
                    nc.tensor.matmul(out=psum_t[:], lhsT=idx_f[:],
                                     rhs=ident[0:1, 0:1], start=True,
                                     stop=True)
                    idx_t = small.tile([8, 1], f32, tag=f"idxt{s}_{r}")
                    nc.vector.tensor_copy(idx_t[:], psum_t[:])
                    psum_e = psum.tile([half, 1], f32, tag=f"pse{s}")
                    nc.tensor.matmul(out=psum_e[:], lhsT=e_mat[0:8, 0:half],
                                     rhs=idx_t[:], start=True, stop=True)
                    idx64_f = small.tile([half, 1], f32, tag=f"i64f{s}_{r}")
                    nc.vector.tensor_scalar(out=idx64_f[:], in0=psum_e[:],
                                            scalar1=float(GR),
                                            scalar2=float(s * C * GR),
                                            op0=Alu.mult, op1=Alu.add)
                    idx64_i = small.tile([half, 1], i32, tag=f"i64i{s}_{r}")
                    nc.vector.tensor_tensor(out=idx64_i[:], in0=idx64_f[:],
                                            in1=a7f[0:half, :], op=Alu.add)
                    # round 0 lands on partitions 0-63 (SDMA engines 0-7),
                    # round 1 on 64-127 (engines 8-15) so the two gathers'
                    # partition-bound descriptors run on disjoint engines
                    nc.gpsimd.indirect_dma_start(
                        out=g[r * half:(r + 1) * half, :], out_offset=None,
                        in_=xg,
                        in_offset=IndirectOffsetOnAxis(ap=idx64_i[:], axis=0))
                    st_eng[r].dma_start(
                        out=y_d[s * GP + r * half:s * GP + (r + 1) * half, :],
                        in_=g[r * half:(r + 1) * half, :])

                nc.vector.max(out=m1[:], in_=w_neg[:])
                nc.vector.max_index(out=idx_u[:, 0:8], in_max=m1[:],
                                    in_values=w_neg[:])
                expand_and_gather(0, m1)
                nc.vector.match_replace(out=w_rep[:], in_to_replace=m1[:],
                                        in_values=w_neg[:], imm_value=-1e38)
                nc.vector.max(out=m2[:], in_=w_rep[:])
                nc.vector.max_index(out=idx_u[:, 8:16], in_max=m2[:],
                                    in_values=w_rep[:])
                expand_and_gather(1, m2)

    nc.compile()
    return nc


def get_nc():
    if "nc" not in _cache:
        _cache["nc"] = _build()
    return _cache["nc"]


def make_in_maps(x: np.ndarray) -> list[dict[str, np.ndarray]]:
    x = np.ascontiguousarray(np.asarray(x, dtype=np.float32))
    assert x.shape == (B, C, H, W)
    return [{"x": x[c * BPC:(c + 1) * BPC].reshape(ROWS, E)}
            for c in range(N_CORES)]


def assemble(results: list[dict[str, np.ndarray]]) -> np.ndarray:
    out = np.empty((B, K, H, W), dtype=np.float32)
    for c in range(N_CORES):
        out[c * BPC:(c + 1) * BPC] = results[c]["y"].reshape(BPC, K, H, W)
    return out


def kernel(x: np.ndarray) -> np.ndarray:
    nc = get_nc()
    res = run_bass_kernel_spmd(nc, make_in_maps(x), list(range(N_CORES)))
    return assemble(res.results)

